# revision 1
# baseline (speedup 1.0000x reference)
"""Causal self-attention (B=4, T=2048, C=768, H=6, D=128) on 8 trn2 NeuronCores.

Sharding: 24 (batch, head) units -> 8 cores, each core owns 1 batch x 3 heads.
Per core: QKV projections for its 3 heads, RoPE + per-head norm, causal
attention, partial output projection over its heads' columns.
Unshard: out[b] = partial[core 2b] + partial[core 2b+1]  (tensor-parallel sum).

Device-side layout choices:
  - Q,K computed in [t, d] layout (rope/norm are free-dim ops), then
    PE-transposed to [d, t] so the scores matmul contracts d on partitions.
  - scores computed TRANSPOSED: sT[s, q] = K[s]:Q[q] so that the AV matmul
    (contraction over s) can consume exp(sT) directly with V in natural [s, d]
    layout; output y arrives as yT[d, q], which is exactly the lhsT layout the
    output projection needs.  No attention-matrix transposes anywhere.
  - softmax denominator: exp tiles accumulated on DVE, partition-summed with a
    ones-vector matmul on PE, reciprocal on DVE, partition-broadcast on GPSIMD.
  - no max-subtraction in softmax: q,k are unit-normalized so |score| <= 11.4
    and exp stays comfortably inside fp32 range (matches reference softmax
    bit-for-bit up to rounding).
  - causal mask inside diagonal blocks: affine_select (iota predicate) zeroes
    exp values where s > q; fully-masked-out blocks are simply never computed.
"""

import numpy as np

import concourse.bacc as bacc
import concourse.bass as bass
import concourse.mybir as mybir
from concourse import tile
from concourse.bass_utils import run_bass_kernel_spmd

F32 = mybir.dt.float32
F32R = mybir.dt.float32r
AF = mybir.ActivationFunctionType
ALU = mybir.AluOpType


# fp32 matmuls run at 4 cycles/row on the PE; float32r (same 4-byte payload,
# different streaming mode) runs at 1 cycle/row for free dim >= 256, so every
# matmul operand tensor below is declared float32r end-to-end.

B, T, C, H, D = 4, 2048, 768, 6, 128
HALF = D // 2
NH = 3            # heads per core
CT = C // 128     # 6 contraction tiles for projections
NT = T // 128     # 16 token tiles
QC = 512          # query-chunk width for attention
NQC = T // QC     # 4 chunks
SCALE = 1.0 / float(np.sqrt(D))
EPS = 1e-6

_CACHE = {}


def _build_nc():
    nc = bacc.Bacc("TRN2")

    xT = nc.dram_tensor("xT", [C, T], F32R, kind="ExternalInput")
    wqT = nc.dram_tensor("wqT", [C, NH * D], F32R, kind="ExternalInput")
    wkT = nc.dram_tensor("wkT", [C, NH * D], F32R, kind="ExternalInput")
    wvT = nc.dram_tensor("wvT", [C, NH * D], F32R, kind="ExternalInput")
    wpT = nc.dram_tensor("wpT", [NH * D, C], F32R, kind="ExternalInput")
    cos3 = nc.dram_tensor("cos3", [T, NH * HALF], F32, kind="ExternalInput")
    sin3 = nc.dram_tensor("sin3", [T, NH * HALF], F32, kind="ExternalInput")
    ident = nc.dram_tensor("ident", [128, 128], F32, kind="ExternalInput")
    ones_in = nc.dram_tensor("ones_in", [128, 1], F32R, kind="ExternalInput")
    out = nc.dram_tensor("out", [T, C], F32, kind="ExternalOutput")

    with tile.TileContext(nc) as tc:
        with (
            tc.tile_pool(name="persist", bufs=1) as persist,
            tc.tile_pool(name="qkvbuf", bufs=1) as qkvbuf,
            tc.tile_pool(name="psA", bufs=3, space="PSUM") as psA,
            tc.tile_pool(name="psY", bufs=3, space="PSUM") as psY,
            tc.tile_pool(name="psT", bufs=2, space="PSUM") as psT,
        ):
            QT = qkvbuf.tile([128, NH, T], F32R)       # [d, h, t]
            KT = qkvbuf.tile([128, NH, T], F32R)       # [d, h, t]
            V = qkvbuf.tile([128, NT, NH * D], F32R)   # [s%128, s//128, h*D+d]
            ones = persist.tile([128, 1], F32R)
            idn = persist.tile([128, 128], F32)
            wp_sb = persist.tile([128, NH, C], F32R)   # [d, h, c]

            # ---------------- stage 1+2: QKV projection + rope + norm ---------
            with (
                tc.tile_pool(name="wbuf", bufs=1) as wbuf,
                tc.tile_pool(name="xch", bufs=3) as xpool,
                tc.tile_pool(name="rope", bufs=3) as rpool,
                tc.tile_pool(name="stat", bufs=6) as spool,
            ):
                wq_sb = wbuf.tile([128, CT, NH * D], F32R)
                wk_sb = wbuf.tile([128, CT, NH * D], F32R)
                wv_sb = wbuf.tile([128, CT, NH * D], F32R)
                # startup-latency ordering: first-tile deps (weights, x tile 0)
                # are issued first; cos/sin next (needed ~us later); wp/ident/
                # ones last (needed only after the first transpose / in stage 3)
                nc.sync.dma_start(wq_sb[:], wqT.rearrange("(ci p) o -> p ci o", p=128))
                nc.sync.dma_start(wk_sb[:], wkT.rearrange("(ci p) o -> p ci o", p=128))
                nc.sync.dma_start(wv_sb[:], wvT.rearrange("(ci p) o -> p ci o", p=128))

                xT_r = xT.rearrange("(ci p) (tt t) -> p ci tt t", p=128, t=128)
                xch0 = xpool.tile([128, CT, 128], F32R, tag="xch")
                nc.sync.dma_start(xch0[:], xT_r[:, :, 0, :])

                cos_sb = wbuf.tile([128, NT, NH * HALF], F32)
                sin_sb = wbuf.tile([128, NT, NH * HALF], F32)
                nc.sync.dma_start(cos_sb[:], cos3.rearrange("(tt p) f -> p tt f", p=128))
                nc.sync.dma_start(sin_sb[:], sin3.rearrange("(tt p) f -> p tt f", p=128))
                nc.sync.dma_start(idn[:], ident[:])
                nc.sync.dma_start(wp_sb[:], wpT.rearrange("(h p) c -> p h c", p=128))
                nc.sync.dma_start(ones[:], ones_in[:])

                for tt in range(NT):
                    if tt == 0:
                        xch = xch0
                    else:
                        xch = xpool.tile([128, CT, 128], F32R, tag="xch")
                        nc.sync.dma_start(xch[:], xT_r[:, :, tt, :])

                    qps = psA.tile([128, NH * D], F32, tag="ps")
                    kps = psA.tile([128, NH * D], F32, tag="ps")
                    vps = psA.tile([128, NH * D], F32, tag="ps")
                    for ci in range(CT):
                        st_, sp_ = (ci == 0), (ci == CT - 1)
                        lhs = xch[:, ci, :]
                        nc.tensor.matmul(qps[:], lhs, wq_sb[:, ci, :], start=st_, stop=sp_)
                        nc.tensor.matmul(kps[:], lhs, wk_sb[:, ci, :], start=st_, stop=sp_)
                        nc.tensor.matmul(vps[:], lhs, wv_sb[:, ci, :], start=st_, stop=sp_)

                    # V: straight copy PSUM -> SBUF in natural [t, o] layout
                    nc.scalar.copy(V[:, tt, :], vps[:])

                    cos_t = cos_sb[:, tt].rearrange("p (h f) -> p h f", h=NH)
                    sin_t = sin_sb[:, tt].rearrange("p (h f) -> p h f", h=NH)

                    # q/k stats share [128, 6] tiles (cols 0-2 q, 3-5 k) so the
                    # tiny per-stat ops run once per token tile, not twice.
                    sums = spool.tile([128, 2 * NH], F32, tag="sums")
                    sumsq = spool.tile([128, 2 * NH], F32, tag="sumsq")
                    rr = []
                    for mi, ps in enumerate((qps, kps)):
                        ps_v = ps[:].rearrange("p (h d) -> p h d", h=NH)
                        a = ps_v[:, :, 0:HALF]      # x1  [128, 3, 64]
                        b = ps_v[:, :, HALF:D]      # x2  [128, 3, 64]
                        r = rpool.tile([128, NH * D], F32, tag=f"r{mi}")
                        rr.append(r)
                        r_v = r[:].rearrange("p (h d) -> p h d", h=NH)
                        r1 = r_v[:, :, 0:HALF]
                        r2 = r_v[:, :, HALF:D]
                        tbs = rpool.tile([128, NH * HALF], F32, tag="tbs")
                        tas = rpool.tile([128, NH * HALF], F32, tag="tas")
                        tbs_v = tbs[:].rearrange("p (h f) -> p h f", h=NH)
                        tas_v = tas[:].rearrange("p (h f) -> p h f", h=NH)
                        # rope: r1 = a*cos + b*sin ; r2 = b*cos - a*sin
                        nc.vector.tensor_mul(tbs_v, b, sin_t)
                        nc.vector.tensor_mul(tas_v, a, sin_t)
                        nc.vector.tensor_mul(r1, a, cos_t)
                        nc.vector.tensor_mul(r2, b, cos_t)
                        nc.vector.tensor_add(r1, r1, tbs_v)
                        nc.vector.tensor_sub(r2, r2, tas_v)

                        sl = slice(mi * NH, (mi + 1) * NH)
                        nc.vector.tensor_reduce(sums[:, sl], r_v, axis=mybir.AxisListType.X, op=ALU.add)
                        sq = rpool.tile([128, NH * D], F32, tag="sq")
                        for h in range(NH):
                            nc.scalar.activation(
                                sq[:, h * D:(h + 1) * D], r[:, h * D:(h + 1) * D],
                                AF.Square, accum_out=sumsq[:, mi * NH + h:mi * NH + h + 1],
                            )
                    # rstd = 1/(sqrt((sumsq - mean*sum)/127) + eps), ddof=1
                    negmean = spool.tile([128, 2 * NH], F32, tag="negmean")
                    nc.scalar.mul(negmean[:], sums[:], -1.0 / D)
                    var = spool.tile([128, 2 * NH], F32, tag="var")
                    nc.vector.tensor_mul(var[:], negmean[:], sums[:])
                    nc.vector.tensor_add(var[:], sumsq[:], var[:])
                    stdv = spool.tile([128, 2 * NH], F32, tag="stdv")
                    nc.scalar.activation(stdv[:], var[:], AF.Sqrt, scale=1.0 / (D - 1))
                    nc.vector.tensor_scalar_add(stdv[:], stdv[:], EPS)
                    rstd = spool.tile([128, 2 * NH], F32, tag="rstd")
                    nc.vector.reciprocal(rstd[:], stdv[:])
                    nmrs = spool.tile([128, 2 * NH], F32, tag="nmrs")
                    nc.vector.tensor_mul(nmrs[:], negmean[:], rstd[:])

                    for mi, dstT in ((0, QT), (1, KT)):
                        r = rr[mi]
                        nrm = rpool.tile([128, NH * D], F32, tag="nrm")
                        tps = psT.tile([128, QC], F32, tag="tp")
                        for h in range(NH):
                            c = mi * NH + h
                            # (r - mean)*rstd = r*rstd + (-mean*rstd)
                            nc.scalar.activation(
                                nrm[:, h * D:(h + 1) * D], r[:, h * D:(h + 1) * D],
                                AF.Identity, bias=nmrs[:, c:c + 1], scale=rstd[:, c:c + 1],
                            )
                            nc.tensor.transpose(tps[:, h * D:(h + 1) * D],
                                                nrm[:, h * D:(h + 1) * D], idn[:])
                        # one strided copy moves all 3 transposed heads out
                        dst = dstT[:, :, tt * 128:(tt + 1) * 128]
                        src = tps[:, 0:NH * D].rearrange("p (h t) -> p h t", h=NH)
                        nc.vector.tensor_copy(dst, src)

            # ---------------- stage 3+4: attention + output projection --------
            with (
                tc.tile_pool(name="att", bufs=3) as apool,
                tc.tile_pool(name="acc", bufs=2) as accpool,
                tc.tile_pool(name="ybuf", bufs=2) as ypool,
                tc.tile_pool(name="obuf", bufs=3) as opool,
            ):
                out_r = out.rearrange("(tt p) c -> p tt c", p=128)
                def emit_proj(qc, yTc):
                    # output projection for chunk qc's 4 token tiles
                    for j in range(QC // 128):
                        tt = qc * (QC // 128) + j
                        op0 = psA.tile([128, C // 2], F32, tag="ps")
                        op1 = psA.tile([128, C // 2], F32, tag="ps")
                        for h in range(NH):
                            lhs = yTc[:, h, j * 128:(j + 1) * 128]
                            nc.tensor.matmul(op0[:], lhs, wp_sb[:, h, 0:C // 2],
                                             start=(h == 0), stop=(h == NH - 1))
                            nc.tensor.matmul(op1[:], lhs, wp_sb[:, h, C // 2:C],
                                             start=(h == 0), stop=(h == NH - 1))
                        ot = opool.tile([128, C], F32, tag="ot")
                        nc.scalar.copy(ot[:, 0:C // 2], op0[:])
                        nc.scalar.copy(ot[:, C // 2:C], op1[:])
                        nc.sync.dma_start(out_r[:, tt, :], ot[:])

                pending = None
                for qc in range(NQC):
                    Q0 = qc * QC
                    n_st = (Q0 + QC) // 128
                    yTc = ypool.tile([128, NH, QC], F32R, tag="yT")  # [d, h, q]
                    for h in range(NH):
                        yps = psY.tile([128, QC], F32, tag="yps")
                        dps = psT.tile([128, QC], F32, tag="tp")
                        for st in range(n_st):
                            sps = psA.tile([128, QC], F32, tag="ps")
                            nc.tensor.matmul(
                                sps[:],
                                KT[:, h, st * 128:(st + 1) * 128],
                                QT[:, h, Q0:Q0 + QC],
                                start=True, stop=True,
                            )
                            et = apool.tile([128, QC], F32R, tag="et")
                            nc.scalar.activation(et[:], sps[:], AF.Exp, scale=SCALE)
                            if st * 128 >= Q0:  # diagonal block: zero where s > q
                                k = st - n_st + 4
                                nc.gpsimd.affine_select(
                                    et[:], et[:],
                                    pattern=[[1, QC]],
                                    compare_op=ALU.is_ge,
                                    fill=0.0,
                                    base=-(128 * k),
                                    channel_multiplier=-1,
                                )
                            nc.tensor.matmul(
                                yps[:],
                                V[:, st, h * D:(h + 1) * D],
                                et[:],
                                start=(st == 0), stop=(st == n_st - 1),
                                skip_group_check=True,
                            )
                            # softmax denominator on PE: accumulate ones^T @ exp
                            nc.tensor.matmul(
                                dps[:1, :],
                                ones[:],
                                et[:],
                                start=(st == 0), stop=(st == n_st - 1),
                                skip_group_check=True,
                            )
                        rc1 = accpool.tile([128, QC], F32, tag="rc1")
                        nc.vector.reciprocal(rc1[:1, :], dps[:1, :])
                        rbc = accpool.tile([128, QC], F32, tag="rbc")
                        nc.gpsimd.partition_broadcast(rbc[:], rc1[:1, :])
                        nc.vector.tensor_mul(yTc[:, h, :], yps[:], rbc[:])
                        if h == 0 and pending is not None:
                            # previous chunk's projection lands here so its
                            # yTc-normalize latency hides under this chunk's
                            # independent attention matmuls
                            emit_proj(*pending)
                            pending = None

                    pending = (qc, yTc)
                emit_proj(*pending)

    nc.compile()
    return nc


def _get_nc():
    if "nc" not in _CACHE:
        _CACHE["nc"] = _build_nc()
    return _CACHE["nc"]


def _in_maps(x, cos, sin, wq, wk, wv, wproj):
    cos3 = np.ascontiguousarray(np.tile(np.asarray(cos, np.float32), (1, NH)))
    sin3 = np.ascontiguousarray(np.tile(np.asarray(sin, np.float32), (1, NH)))
    ident = np.eye(128, dtype=np.float32)
    maps = []
    for c in range(8):
        b = c // 2
        hs = (c % 2) * NH
        sl = slice(hs * D, (hs + NH) * D)
        maps.append({
            "xT": np.ascontiguousarray(np.asarray(x[b], np.float32).T),
            "wqT": np.ascontiguousarray(np.asarray(wq, np.float32)[sl].T),
            "wkT": np.ascontiguousarray(np.asarray(wk, np.float32)[sl].T),
            "wvT": np.ascontiguousarray(np.asarray(wv, np.float32)[sl].T),
            "wpT": np.ascontiguousarray(np.asarray(wproj, np.float32).T[sl]),
            "cos3": cos3,
            "sin3": sin3,
            "ident": ident,
            "ones_in": np.ones((128, 1), dtype=np.float32),
        })
    return maps


def kernel(x, cos, sin, wq, wk, wv, wproj, _trace=False):
    nc = _get_nc()
    maps = _in_maps(x, cos, sin, wq, wk, wv, wproj)
    res = run_bass_kernel_spmd(nc, maps, core_ids=list(range(8)), trace=_trace)
    parts = [r["out"] for r in res.results]
    outv = np.stack([parts[2 * b] + parts[2 * b + 1] for b in range(B)]).astype(np.float32)
    if _trace:
        _CACHE["last_results"] = res
    return outv



# revision 11
# speedup vs baseline: 1.0168x; 1.0168x over previous
"""Causal self-attention (B=4, T=2048, C=768, H=6, D=128) on 8 trn2 NeuronCores.

Sharding: 24 (batch, head) units -> 8 cores, each core owns 1 batch x 3 heads.
Unshard: out[b] = partial[core 2b] + partial[core 2b+1]  (tensor-parallel sum).

v2 design notes (vs the fp32r baseline):
  - bf16 on every PE operand (weights, x, Q, K, V, exp(scores), wproj): PE
    streams 1 cycle/row at any free-dim, transposes drop 2.0->1.0 cyc/row,
    DMA volume halves, and DVE gets its 2x/4x 16-bit perf modes.
  - stage 1 rebalanced off the Scalar engine (it was 68% busy): rope runs as
    4 big grouped ops (2 on DVE, 2 on Pool) with stride-0 broadcast cos/sin
    views; mean/var stats via grouped tensor_reduce; normalize via fused
    (r + negmean)*rstd tensor_scalar on Pool/DVE; only the V-copy, psT
    copies and sqrt stay on Scalar.
  - attention: causal diagonal blocks compute only the live column range
    (free dims 512/384/256/128); the in-diagonal mask is a host-supplied
    [128,512] lower-triangle bf16 tile applied as one DVE multiply.
  - softmax denominators for the 3 heads accumulate into partitions 0/32/64
    of one PSUM tile via 1-row ones-matmuls; one reciprocal per chunk.
  - PE kept continuously busy (pstate ramps to 2.4GHz after 3us of
    uninterrupted work): scores prefetched 2 blocks ahead of AV, stage-1
    transposes lag their tile by one iteration, projection of chunk qc-1
    is emitted behind chunk qc's first score block.
"""

import numpy as np
import ml_dtypes

import concourse.bacc as bacc
import concourse.bass as bass
import concourse.mybir as mybir
from concourse import tile
from concourse.bass_utils import run_bass_kernel_spmd

F32 = mybir.dt.float32
F32R = mybir.dt.float32r
BF16 = mybir.dt.bfloat16
AF = mybir.ActivationFunctionType
ALU = mybir.AluOpType

B, T, C, H, D = 4, 2048, 768, 6, 128
HALF = D // 2
NH = 3            # heads per core
CT = C // 128     # 6 contraction tiles for projections
NT = T // 128     # 16 token tiles
QC = 512          # query-chunk width for attention
NQC = T // QC     # 4 chunks
SCALE = 1.0 / float(np.sqrt(D))
EPS = 1e-6

_CACHE = {}


def _build_nc():
    nc = bacc.Bacc("TRN2")

    xT = nc.dram_tensor("xT", [C, T], BF16, kind="ExternalInput")
    wqT = nc.dram_tensor("wqT", [C, NH * D], BF16, kind="ExternalInput")
    wkT = nc.dram_tensor("wkT", [C, NH * D], BF16, kind="ExternalInput")
    wvT = nc.dram_tensor("wvT", [C, NH * D], BF16, kind="ExternalInput")
    wpT = nc.dram_tensor("wpT", [NH * D, C], BF16, kind="ExternalInput")
    cosb = nc.dram_tensor("cosb", [T, HALF], BF16, kind="ExternalInput")
    sinb = nc.dram_tensor("sinb", [T, HALF], BF16, kind="ExternalInput")
    maskC = nc.dram_tensor("maskC", [128, QC], BF16, kind="ExternalInput")
    ident = nc.dram_tensor("ident", [128, 128], F32R, kind="ExternalInput")
    ones_in = nc.dram_tensor("ones_in", [128, 1], BF16, kind="ExternalInput")
    out = nc.dram_tensor("out", [T, C], F32, kind="ExternalOutput")

    with tile.TileContext(nc) as tc:
        with (
            tc.tile_pool(name="persist", bufs=1) as persist,
            tc.tile_pool(name="qkvbuf", bufs=1) as qkvbuf,
        ):
            QT = qkvbuf.tile([128, NH, T], BF16)       # [d, h, t]
            KT = qkvbuf.tile([128, NH, T], BF16)       # [d, h, t]
            V = qkvbuf.tile([128, NT, NH * D], BF16)   # [s%128, s//128, h*D+d]
            ones = persist.tile([128, 1], BF16)
            idn = persist.tile([128, 128], F32R)
            mask = persist.tile([128, QC], BF16)
            wp_sb = persist.tile([128, NH, C], BF16)   # [d, h, c]

            # ---------------- stage 1+2: QKV projection + rope + norm ---------
            with (
                tc.tile_pool(name="wbuf", bufs=1) as wbuf,
                tc.tile_pool(name="xch", bufs=3) as xpool,
                tc.tile_pool(name="rope", bufs=3) as rpool,
                tc.tile_pool(name="nrmp", bufs=3) as npool,
                tc.tile_pool(name="stat", bufs=4) as spool,
                tc.tile_pool(name="psQKV", bufs=2, space="PSUM") as psQKV,
                tc.tile_pool(name="psT", bufs=2, space="PSUM") as psT,
            ):
                wq_sb = wbuf.tile([128, CT, NH * D], BF16)
                wk_sb = wbuf.tile([128, CT, NH * D], BF16)
                wv_sb = wbuf.tile([128, CT, NH * D], BF16)
                # startup ordering: first-matmul deps (wq, x tile 0) land
                # first, the rest in first-use order.
                nc.sync.dma_start(wq_sb[:], wqT.rearrange("(ci p) o -> p ci o", p=128))

                xT_r = xT.rearrange("(ci p) (tt t) -> p ci tt t", p=128, t=128)
                xch0 = xpool.tile([128, CT, 128], BF16, tag="xch")
                nc.sync.dma_start(xch0[:], xT_r[:, :, 0, :])

                nc.sync.dma_start(wk_sb[:], wkT.rearrange("(ci p) o -> p ci o", p=128))
                nc.sync.dma_start(wv_sb[:], wvT.rearrange("(ci p) o -> p ci o", p=128))

                cos_sb = wbuf.tile([128, NT, HALF], BF16)
                sin_sb = wbuf.tile([128, NT, HALF], BF16)
                nc.sync.dma_start(cos_sb[:], cosb.rearrange("(tt p) f -> p tt f", p=128))
                nc.sync.dma_start(sin_sb[:], sinb.rearrange("(tt p) f -> p tt f", p=128))
                nc.sync.dma_start(idn[:], ident[:])
                nc.sync.dma_start(mask[:], maskC[:])
                nc.sync.dma_start(ones[:], ones_in[:])
                nc.sync.dma_start(wp_sb[:], wpT.rearrange("(h p) c -> p h c", p=128))

                # software pipeline: transposes of tile tt-1 are emitted after
                # the projections of tile tt so the PE never waits on the
                # vector-engine norm chain.
                pend_tp = None

                def emit_transposes(nrm, tt):
                    for mi in range(2):
                        dstT = QT if mi == 0 else KT
                        tps = psT.tile([128, NH * D], F32R, tag="tp")
                        for h in range(NH):
                            nc.tensor.transpose(
                                tps[:, h * D:(h + 1) * D], nrm[:, mi, h], idn[:])
                        # one strided copy moves all 3 transposed heads out
                        dst = dstT[:, :, tt * 128:(tt + 1) * 128]
                        src = tps[:].rearrange("p (h t) -> p h t", h=NH)
                        nc.scalar.copy(dst, src)

                for tt in range(NT):
                    if tt == 0:
                        xch = xch0
                    else:
                        xch = xpool.tile([128, CT, 128], BF16, tag="xch")
                        nc.sync.dma_start(xch[:], xT_r[:, :, tt, :])

                    # q in bank 0, k in bank 1, v in bank 2 of one psum tile
                    qkv = psQKV.tile([128, 3, 512], F32, tag="qkv")
                    for ci in range(CT):
                        st_, sp_ = (ci == 0), (ci == CT - 1)
                        lhs = xch[:, ci, :]
                        nc.tensor.matmul(qkv[:, 0, 0:NH * D], lhs, wq_sb[:, ci, :],
                                         start=st_, stop=sp_, skip_group_check=True)
                        nc.tensor.matmul(qkv[:, 1, 0:NH * D], lhs, wk_sb[:, ci, :],
                                         start=st_, stop=sp_, skip_group_check=True)
                        nc.tensor.matmul(qkv[:, 2, 0:NH * D], lhs, wv_sb[:, ci, :],
                                         start=st_, stop=sp_, skip_group_check=True)

                    if pend_tp is not None:
                        emit_transposes(*pend_tp)
                        pend_tp = None

                    # V: straight copy PSUM -> SBUF (bf16) in natural [t, o]
                    nc.scalar.copy(V[:, tt, :], qkv[:, 2, 0:NH * D])

                    # ---- rope: r = t (.) [c,c]  +  [b,a] (.) [s,-s] --------
                    # Pool cannot read PSUM, so ACT (fast PSUM reader) parks
                    # q,k in SBUF bf16 first; everything downstream then runs
                    # in the 16-bit fast modes.
                    qk_sb = rpool.tile([128, 2, NH, D], BF16, tag="qksb")
                    nc.scalar.copy(qk_sb[:].rearrange("p m h f -> p m (h f)"),
                                   qkv[:, 0:2, 0:NH * D])
                    # BIR vector ops want <=3D APs: work on [p, (m h), f] views
                    qk6 = qk_sb[:].rearrange("p m h f -> p (m h) f")
                    a = qk6[:, :, 0:HALF]
                    b = qk6[:, :, HALF:D]
                    cos_b3 = cos_sb[:, tt].unsqueeze(1).broadcast_to((128, 2 * NH, HALF))
                    sin_b3 = sin_sb[:, tt].unsqueeze(1).broadcast_to((128, 2 * NH, HALF))
                    u = rpool.tile([128, 2, NH, D], BF16, tag="u")
                    w = rpool.tile([128, 2, NH, D], BF16, tag="w")
                    r = rpool.tile([128, 2, NH, D], BF16, tag="r")
                    u6 = u[:].rearrange("p m h f -> p (m h) f")
                    w6 = w[:].rearrange("p m h f -> p (m h) f")
                    # u = [a*c, b*c] and w0 = b*s: plain TTs on Pool (the Pool
                    # engine rejects TensorScalarPtr, so those stay on DVE)
                    nc.gpsimd.tensor_tensor(u6[:, :, 0:HALF], a, cos_b3, op=ALU.mult)
                    nc.gpsimd.tensor_tensor(u6[:, :, HALF:D], b, cos_b3, op=ALU.mult)
                    nc.gpsimd.tensor_tensor(w6[:, :, 0:HALF], b, sin_b3, op=ALU.mult)
                    # w1 = -a*s  (DVE scalar_tensor_tensor)
                    nc.vector.scalar_tensor_tensor(
                        w6[:, :, HALF:D], a, -1.0, sin_b3, op0=ALU.mult, op1=ALU.mult)
                    # r = u + w   (DVE, bf16 fast mode)
                    nc.vector.tensor_add(r[:], u[:], w[:])

                    # ---- stats: mean / unbiased std per (token, head) ------
                    r6 = r[:].rearrange("p m h f -> p (m h) f")
                    sums = spool.tile([128, 2 * NH], F32, tag="sums")
                    nc.vector.tensor_reduce(sums[:], r6, axis=mybir.AxisListType.X, op=ALU.add)
                    junk = rpool.tile([128, 2, NH, D], BF16, tag="junk")
                    nc.gpsimd.tensor_mul(junk[:], r[:], r[:])
                    sumsq = spool.tile([128, 2 * NH], F32, tag="sumsq")
                    nc.vector.tensor_reduce(
                        sumsq[:], junk[:].rearrange("p m h f -> p (m h) f"),
                        axis=mybir.AxisListType.X, op=ALU.add)
                    negmean = spool.tile([128, 2 * NH], F32, tag="negmean")
                    nc.vector.tensor_scalar_mul(negmean[:], sums[:], -1.0 / D)
                    var = spool.tile([128, 2 * NH], F32, tag="var")
                    nc.gpsimd.tensor_mul(var[:], negmean[:], sums[:])
                    nc.gpsimd.tensor_add(var[:], sumsq[:], var[:])
                    stdv = spool.tile([128, 2 * NH], F32, tag="stdv")
                    nc.scalar.activation(stdv[:], var[:], AF.Sqrt, scale=1.0 / (D - 1))
                    nc.vector.tensor_scalar_add(stdv[:], stdv[:], EPS)
                    rstd = spool.tile([128, 2 * NH], F32, tag="rstd")
                    nc.vector.reciprocal(rstd[:], stdv[:])

                    # ---- normalize: nrm = (r + negmean) * rstd -------------
                    nrm = npool.tile([128, 2, NH, D], F32R, tag="nrm")
                    for mi in range(2):
                        for h in range(NH):
                            c = mi * NH + h
                            nc.vector.tensor_scalar(
                                nrm[:, mi, h], r[:, mi, h],
                                negmean[:, c:c + 1], rstd[:, c:c + 1],
                                op0=ALU.add, op1=ALU.mult)

                    pend_tp = (nrm, tt)

                emit_transposes(*pend_tp)

            # ---------------- stage 3+4: attention + output projection --------
            with (
                tc.tile_pool(name="att", bufs=4) as apool,
                tc.tile_pool(name="acc", bufs=2) as accpool,
                tc.tile_pool(name="ybuf", bufs=2) as ypool,
                tc.tile_pool(name="obuf", bufs=3) as opool,
                tc.tile_pool(name="psS", bufs=4, space="PSUM") as psS,
                tc.tile_pool(name="psY", bufs=2, space="PSUM") as psY,
                tc.tile_pool(name="psD", bufs=2, space="PSUM") as psD,
            ):
                out_r = out.rearrange("(tt p) c -> p tt c", p=128)

                def emit_proj(qc, yTc):
                    # output projection for chunk qc's 4 token tiles
                    for j in range(QC // 128):
                        tt = qc * (QC // 128) + j
                        op0 = psS.tile([128, C // 2], F32, tag="ps")
                        op1 = psS.tile([128, C // 2], F32, tag="ps")
                        for h in range(NH):
                            lhs = yTc[:, h, j * 128:(j + 1) * 128]
                            nc.tensor.matmul(op0[:], lhs, wp_sb[:, h, 0:C // 2],
                                             start=(h == 0), stop=(h == NH - 1))
                            nc.tensor.matmul(op1[:], lhs, wp_sb[:, h, C // 2:C],
                                             start=(h == 0), stop=(h == NH - 1))
                        ot = opool.tile([128, C], F32, tag="ot")
                        nc.scalar.copy(ot[:, 0:C // 2], op0[:])
                        nc.scalar.copy(ot[:, C // 2:C], op1[:])
                        nc.sync.dma_start(out_r[:, tt, :], ot[:])

                pending = None
                for qc in range(NQC):
                    Q0 = qc * QC
                    n_st = (Q0 + QC) // 128
                    dps = psD.tile([128, QC], F32, tag="dps")
                    yU = ypool.tile([128, NH, QC], BF16, tag="yU")
                    ets = {}

                    def loc0_of(st):
                        # first live column (within the chunk) of block st
                        j = st - (n_st - 4)
                        return 128 * j if j > 0 else 0

                    def emit_score(h, st):
                        loc0 = loc0_of(st)
                        sps = psS.tile([128, QC], F32, tag="ps")
                        nc.tensor.matmul(
                            sps[:, loc0:QC],
                            KT[:, h, st * 128:(st + 1) * 128],
                            QT[:, h, Q0 + loc0:Q0 + QC],
                            start=True, stop=True)
                        et = apool.tile([128, QC], BF16, tag="et")
                        nc.scalar.activation(et[:, loc0:QC], sps[:, loc0:QC],
                                             AF.Exp, scale=SCALE)
                        if st * 128 >= Q0:  # diagonal block: zero where s > q
                            nc.vector.tensor_mul(et[:, loc0:QC], et[:, loc0:QC],
                                                 mask[:, 0:QC - loc0])
                        ets[(h, st)] = et

                    def emit_av(h, st, yps):
                        loc0 = loc0_of(st)
                        et = ets.pop((h, st))
                        nc.tensor.matmul(
                            yps[:, loc0:QC],
                            V[:, st, h * D:(h + 1) * D],
                            et[:, loc0:QC],
                            start=(st == 0), stop=(st == n_st - 1),
                            skip_group_check=True)
                        nc.tensor.matmul(
                            dps[32 * h:32 * h + 1, loc0:QC],
                            ones[:],
                            et[:, loc0:QC],
                            start=(st == 0), stop=(st == n_st - 1),
                            skip_group_check=True)

                    for h in range(NH):
                        yps = psY.tile([128, QC], F32, tag="yps")
                        emit_score(h, 0)
                        if n_st > 1:
                            emit_score(h, 1)
                        for st in range(n_st):
                            if st + 2 < n_st:
                                emit_score(h, st + 2)
                            emit_av(h, st, yps)
                        if h == 0 and pending is not None:
                            # previous chunk's projection: its yTc normalize
                            # latency hides under this chunk's score matmuls
                            emit_proj(*pending)
                            pending = None
                        # park unnormalized y in SBUF so the psum bank frees
                        nc.vector.tensor_copy(yU[:, h, :], yps[:])

                    # softmax denominators: heads live at partitions 0/32/64;
                    # one reciprocal covers all three (rows between are unused
                    # garbage, reciprocal of garbage is harmless).
                    rc = accpool.tile([128, QC], BF16, tag="rc")
                    with nc.allow_low_precision(reason="bf16 softmax denom"):
                        nc.vector.reciprocal(rc[0:65, :], dps[0:65, :])
                    yTc = ypool.tile([128, NH, QC], BF16, tag="yT")  # [d, h, q]
                    for h in range(NH):
                        # partition_broadcast is only safe from base partition
                        # 0 — stage each head's row there first
                        rch = accpool.tile([1, QC], BF16, tag="rch")
                        nc.vector.tensor_copy(rch[:], rc[32 * h:32 * h + 1, :])
                        rbc = accpool.tile([128, QC], BF16, tag="rbc")
                        nc.gpsimd.partition_broadcast(rbc[:], rch[:])
                        nc.vector.tensor_mul(yTc[:, h, :], yU[:, h, :], rbc[:])

                    pending = (qc, yTc)
                emit_proj(*pending)

    nc.compile()
    return nc


def _get_nc():
    if "nc" not in _CACHE:
        _CACHE["nc"] = _build_nc()
    return _CACHE["nc"]


def _in_maps(x, cos, sin, wq, wk, wv, wproj):
    bf = ml_dtypes.bfloat16
    cosb = np.ascontiguousarray(np.asarray(cos).astype(bf))
    sinb = np.ascontiguousarray(np.asarray(sin).astype(bf))
    maskC = np.ascontiguousarray(
        (np.arange(QC)[None, :] >= np.arange(128)[:, None]).astype(bf))
    ident = np.eye(128, dtype=np.float32)
    maps = []
    for c in range(8):
        b = c // 2
        hs = (c % 2) * NH
        sl = slice(hs * D, (hs + NH) * D)
        maps.append({
            "xT": np.ascontiguousarray(np.asarray(x[b]).T.astype(bf)),
            "wqT": np.ascontiguousarray(np.asarray(wq)[sl].T.astype(bf)),
            "wkT": np.ascontiguousarray(np.asarray(wk)[sl].T.astype(bf)),
            "wvT": np.ascontiguousarray(np.asarray(wv)[sl].T.astype(bf)),
            "wpT": np.ascontiguousarray(np.asarray(wproj).T[sl].astype(bf)),
            "cosb": cosb,
            "sinb": sinb,
            "maskC": maskC,
            "ident": ident,
            "ones_in": np.ones((128, 1), dtype=bf),
        })
    return maps


def kernel(x, cos, sin, wq, wk, wv, wproj, _trace=False):
    nc = _get_nc()
    maps = _in_maps(x, cos, sin, wq, wk, wv, wproj)
    res = run_bass_kernel_spmd(nc, maps, core_ids=list(range(8)), trace=_trace)
    parts = [r["out"] for r in res.results]
    outv = np.stack([parts[2 * b] + parts[2 * b + 1] for b in range(B)]).astype(np.float32)
    if _trace:
        _CACHE["last_results"] = res
    return outv


# revision 12
# speedup vs baseline: 1.0258x; 1.0089x over previous
"""Causal self-attention (B=4, T=2048, C=768, H=6, D=128) on 8 trn2 NeuronCores.

Sharding: 24 (batch, head) units -> 8 cores, each core owns 1 batch x 3 heads.
Unshard: out[b] = partial[core 2b] + partial[core 2b+1]  (tensor-parallel sum).

v2 design notes (vs the fp32r baseline):
  - bf16 on every PE operand (weights, x, Q, K, V, exp(scores), wproj): PE
    streams 1 cycle/row at any free-dim, transposes drop 2.0->1.0 cyc/row,
    DMA volume halves, and DVE gets its 2x/4x 16-bit perf modes.
  - stage 1 rebalanced off the Scalar engine (it was 68% busy): rope runs as
    4 big grouped ops (2 on DVE, 2 on Pool) with stride-0 broadcast cos/sin
    views; mean/var stats via grouped tensor_reduce; normalize via fused
    (r + negmean)*rstd tensor_scalar on Pool/DVE; only the V-copy, psT
    copies and sqrt stay on Scalar.
  - attention: causal diagonal blocks compute only the live column range
    (free dims 512/384/256/128); the in-diagonal mask is a host-supplied
    [128,512] lower-triangle bf16 tile applied as one DVE multiply.
  - softmax denominators for the 3 heads accumulate into partitions 0/32/64
    of one PSUM tile via 1-row ones-matmuls; one reciprocal per chunk.
  - PE kept continuously busy (pstate ramps to 2.4GHz after 3us of
    uninterrupted work): scores prefetched 2 blocks ahead of AV, stage-1
    transposes lag their tile by one iteration, projection of chunk qc-1
    is emitted behind chunk qc's first score block.
"""

import numpy as np
import ml_dtypes

import concourse.bacc as bacc
import concourse.bass as bass
import concourse.mybir as mybir
from concourse import tile
from concourse.bass_utils import run_bass_kernel_spmd

F32 = mybir.dt.float32
F32R = mybir.dt.float32r
BF16 = mybir.dt.bfloat16
AF = mybir.ActivationFunctionType
ALU = mybir.AluOpType

B, T, C, H, D = 4, 2048, 768, 6, 128
HALF = D // 2
NH = 3            # heads per core
CT = C // 128     # 6 contraction tiles for projections
NT = T // 128     # 16 token tiles
QC = 512          # query-chunk width for attention
NQC = T // QC     # 4 chunks
SCALE = 1.0 / float(np.sqrt(D))
EPS = 1e-6

_CACHE = {}


def _build_nc():
    nc = bacc.Bacc("TRN2")

    xT = nc.dram_tensor("xT", [C, T], BF16, kind="ExternalInput")
    wqT = nc.dram_tensor("wqT", [C, NH * D], BF16, kind="ExternalInput")
    wkT = nc.dram_tensor("wkT", [C, NH * D], BF16, kind="ExternalInput")
    wvT = nc.dram_tensor("wvT", [C, NH * D], BF16, kind="ExternalInput")
    wpT = nc.dram_tensor("wpT", [NH * D, C], BF16, kind="ExternalInput")
    cosb = nc.dram_tensor("cosb", [T, HALF], BF16, kind="ExternalInput")
    sinb = nc.dram_tensor("sinb", [T, HALF], BF16, kind="ExternalInput")
    maskC = nc.dram_tensor("maskC", [128, QC], BF16, kind="ExternalInput")
    ident = nc.dram_tensor("ident", [128, 128], BF16, kind="ExternalInput")
    ones_in = nc.dram_tensor("ones_in", [128, 1], BF16, kind="ExternalInput")
    out = nc.dram_tensor("out", [T, C], F32, kind="ExternalOutput")

    with tile.TileContext(nc) as tc:
        with (
            tc.tile_pool(name="persist", bufs=1) as persist,
            tc.tile_pool(name="qkvbuf", bufs=1) as qkvbuf,
        ):
            QT = qkvbuf.tile([128, NH, T], BF16)       # [d, h, t]
            KT = qkvbuf.tile([128, NH, T], BF16)       # [d, h, t]
            V = qkvbuf.tile([128, NT, NH * D], BF16)   # [s%128, s//128, h*D+d]
            ones = persist.tile([128, 1], BF16)
            idn = persist.tile([128, 128], BF16)
            mask = persist.tile([128, QC], BF16)
            wp_sb = persist.tile([128, NH, C], BF16)   # [d, h, c]

            # ---------------- stage 1+2: QKV projection + rope + norm ---------
            with (
                tc.tile_pool(name="wbuf", bufs=1) as wbuf,
                tc.tile_pool(name="xch", bufs=3) as xpool,
                tc.tile_pool(name="rope", bufs=3) as rpool,
                tc.tile_pool(name="nrmp", bufs=3) as npool,
                tc.tile_pool(name="stat", bufs=4) as spool,
                tc.tile_pool(name="psQKV", bufs=2, space="PSUM") as psQKV,
                tc.tile_pool(name="psT", bufs=2, space="PSUM") as psT,
            ):
                wq_sb = wbuf.tile([128, CT, NH * D], BF16)
                wk_sb = wbuf.tile([128, CT, NH * D], BF16)
                wv_sb = wbuf.tile([128, CT, NH * D], BF16)
                # startup ordering: first-matmul deps (wq, x tile 0) land
                # first, the rest in first-use order.
                wqT_r = wqT.rearrange("(ci p) o -> p ci o", p=128)
                nc.sync.dma_start(wq_sb[:, 0:2], wqT_r[:, 0:2])

                xT_r = xT.rearrange("(ci p) (tt t) -> p ci tt t", p=128, t=128)
                xch0 = xpool.tile([128, CT, 128], BF16, tag="xch")
                nc.sync.dma_start(xch0[:], xT_r[:, :, 0, :])
                nc.sync.dma_start(wq_sb[:, 2:CT], wqT_r[:, 2:CT])

                nc.sync.dma_start(wk_sb[:], wkT.rearrange("(ci p) o -> p ci o", p=128))
                nc.sync.dma_start(wv_sb[:], wvT.rearrange("(ci p) o -> p ci o", p=128))

                cos_sb = wbuf.tile([128, NT, HALF], BF16)
                sin_sb = wbuf.tile([128, NT, HALF], BF16)
                nc.sync.dma_start(cos_sb[:], cosb.rearrange("(tt p) f -> p tt f", p=128))
                nc.sync.dma_start(sin_sb[:], sinb.rearrange("(tt p) f -> p tt f", p=128))
                nc.sync.dma_start(idn[:], ident[:])
                nc.sync.dma_start(mask[:], maskC[:])
                nc.sync.dma_start(ones[:], ones_in[:])
                nc.sync.dma_start(wp_sb[:], wpT.rearrange("(h p) c -> p h c", p=128))

                # software pipeline: transposes of tile tt-1 are emitted after
                # the projections of tile tt so the PE never waits on the
                # vector-engine norm chain.
                pend_tp = None

                def emit_transposes(nrm, tt):
                    for mi in range(2):
                        dstT = QT if mi == 0 else KT
                        tps = psT.tile([128, NH * D], BF16, tag="tp")
                        for h in range(NH):
                            nc.tensor.transpose(
                                tps[:, h * D:(h + 1) * D], nrm[:, mi, h], idn[:])
                        # one strided copy moves all 3 transposed heads out
                        dst = dstT[:, :, tt * 128:(tt + 1) * 128]
                        src = tps[:].rearrange("p (h t) -> p h t", h=NH)
                        nc.scalar.copy(dst, src)

                for tt in range(NT):
                    if tt == 0:
                        xch = xch0
                    else:
                        xch = xpool.tile([128, CT, 128], BF16, tag="xch")
                        nc.sync.dma_start(xch[:], xT_r[:, :, tt, :])

                    # q in bank 0, k in bank 1, v in bank 2 of one psum tile
                    qkv = psQKV.tile([128, 3, 512], F32, tag="qkv")
                    for ci in range(CT):
                        st_, sp_ = (ci == 0), (ci == CT - 1)
                        lhs = xch[:, ci, :]
                        nc.tensor.matmul(qkv[:, 0, 0:NH * D], lhs, wq_sb[:, ci, :],
                                         start=st_, stop=sp_, skip_group_check=True)
                        nc.tensor.matmul(qkv[:, 1, 0:NH * D], lhs, wk_sb[:, ci, :],
                                         start=st_, stop=sp_, skip_group_check=True)
                        nc.tensor.matmul(qkv[:, 2, 0:NH * D], lhs, wv_sb[:, ci, :],
                                         start=st_, stop=sp_, skip_group_check=True)

                    if pend_tp is not None:
                        emit_transposes(*pend_tp)
                        pend_tp = None

                    # V: straight copy PSUM -> SBUF (bf16) in natural [t, o]
                    nc.scalar.copy(V[:, tt, :], qkv[:, 2, 0:NH * D])

                    # ---- rope: r = t (.) [c,c]  +  [b,a] (.) [s,-s] --------
                    # Pool cannot read PSUM, so ACT (fast PSUM reader) parks
                    # q,k in SBUF bf16 first; everything downstream then runs
                    # in the 16-bit fast modes.
                    qk_sb = rpool.tile([128, 2, NH, D], BF16, tag="qksb")
                    nc.scalar.copy(qk_sb[:].rearrange("p m h f -> p m (h f)"),
                                   qkv[:, 0:2, 0:NH * D])
                    # BIR vector ops want <=3D APs: work on [p, (m h), f] views
                    qk6 = qk_sb[:].rearrange("p m h f -> p (m h) f")
                    a = qk6[:, :, 0:HALF]
                    b = qk6[:, :, HALF:D]
                    cos_b3 = cos_sb[:, tt].unsqueeze(1).broadcast_to((128, 2 * NH, HALF))
                    sin_b3 = sin_sb[:, tt].unsqueeze(1).broadcast_to((128, 2 * NH, HALF))
                    u = rpool.tile([128, 2, NH, D], BF16, tag="u")
                    w = rpool.tile([128, 2, NH, D], BF16, tag="w")
                    r = rpool.tile([128, 2, NH, D], BF16, tag="r")
                    u6 = u[:].rearrange("p m h f -> p (m h) f")
                    w6 = w[:].rearrange("p m h f -> p (m h) f")
                    # u = [a*c, b*c] and w0 = b*s: plain TTs on Pool (the Pool
                    # engine rejects TensorScalarPtr, so those stay on DVE)
                    nc.gpsimd.tensor_tensor(u6[:, :, 0:HALF], a, cos_b3, op=ALU.mult)
                    nc.gpsimd.tensor_tensor(u6[:, :, HALF:D], b, cos_b3, op=ALU.mult)
                    nc.gpsimd.tensor_tensor(w6[:, :, 0:HALF], b, sin_b3, op=ALU.mult)
                    # w1 = -a*s  (DVE scalar_tensor_tensor)
                    nc.vector.scalar_tensor_tensor(
                        w6[:, :, HALF:D], a, -1.0, sin_b3, op0=ALU.mult, op1=ALU.mult)
                    # r = u + w   (DVE, bf16 fast mode)
                    nc.vector.tensor_add(r[:], u[:], w[:])

                    # ---- stats: mean / unbiased std per (token, head) ------
                    r6 = r[:].rearrange("p m h f -> p (m h) f")
                    sums = spool.tile([128, 2 * NH], F32, tag="sums")
                    nc.vector.tensor_reduce(sums[:], r6, axis=mybir.AxisListType.X, op=ALU.add)
                    junk = rpool.tile([128, 2, NH, D], BF16, tag="junk")
                    nc.gpsimd.tensor_mul(junk[:], r[:], r[:])
                    sumsq = spool.tile([128, 2 * NH], F32, tag="sumsq")
                    nc.vector.tensor_reduce(
                        sumsq[:], junk[:].rearrange("p m h f -> p (m h) f"),
                        axis=mybir.AxisListType.X, op=ALU.add)
                    negmean = spool.tile([128, 2 * NH], F32, tag="negmean")
                    nc.vector.tensor_scalar_mul(negmean[:], sums[:], -1.0 / D)
                    var = spool.tile([128, 2 * NH], F32, tag="var")
                    nc.gpsimd.tensor_mul(var[:], negmean[:], sums[:])
                    nc.gpsimd.tensor_add(var[:], sumsq[:], var[:])
                    stdv = spool.tile([128, 2 * NH], F32, tag="stdv")
                    nc.scalar.activation(stdv[:], var[:], AF.Sqrt, scale=1.0 / (D - 1))
                    nc.vector.tensor_scalar_add(stdv[:], stdv[:], EPS)
                    rstd = spool.tile([128, 2 * NH], F32, tag="rstd")
                    nc.vector.reciprocal(rstd[:], stdv[:])

                    # ---- normalize: nrm = (r + negmean) * rstd -------------
                    nrm = npool.tile([128, 2, NH, D], BF16, tag="nrm")
                    for mi in range(2):
                        for h in range(NH):
                            c = mi * NH + h
                            nc.vector.tensor_scalar(
                                nrm[:, mi, h], r[:, mi, h],
                                negmean[:, c:c + 1], rstd[:, c:c + 1],
                                op0=ALU.add, op1=ALU.mult)

                    pend_tp = (nrm, tt)

                emit_transposes(*pend_tp)

            # ---------------- stage 3+4: attention + output projection --------
            with (
                tc.tile_pool(name="att", bufs=5) as apool,
                tc.tile_pool(name="acc", bufs=2) as accpool,
                tc.tile_pool(name="ybuf", bufs=2) as ypool,
                tc.tile_pool(name="obuf", bufs=3) as opool,
                tc.tile_pool(name="psS", bufs=4, space="PSUM") as psS,
                tc.tile_pool(name="psY", bufs=2, space="PSUM") as psY,
                tc.tile_pool(name="psD", bufs=2, space="PSUM") as psD,
            ):
                out_r = out.rearrange("(tt p) c -> p tt c", p=128)

                def emit_proj(qc, yTc):
                    # output projection for chunk qc's 4 token tiles
                    for j in range(QC // 128):
                        tt = qc * (QC // 128) + j
                        op0 = psS.tile([128, C // 2], F32, tag="ps")
                        op1 = psS.tile([128, C // 2], F32, tag="ps")
                        for h in range(NH):
                            lhs = yTc[:, h, j * 128:(j + 1) * 128]
                            nc.tensor.matmul(op0[:], lhs, wp_sb[:, h, 0:C // 2],
                                             start=(h == 0), stop=(h == NH - 1))
                            nc.tensor.matmul(op1[:], lhs, wp_sb[:, h, C // 2:C],
                                             start=(h == 0), stop=(h == NH - 1))
                        ot = opool.tile([128, C], F32, tag="ot")
                        nc.scalar.copy(ot[:, 0:C // 2], op0[:])
                        nc.scalar.copy(ot[:, C // 2:C], op1[:])
                        nc.sync.dma_start(out_r[:, tt, :], ot[:])

                pending = None
                for qc in range(NQC):
                    Q0 = qc * QC
                    n_st = (Q0 + QC) // 128
                    dps = psD.tile([128, QC], F32, tag="dps")
                    yU = ypool.tile([128, NH, QC], BF16, tag="yU")
                    ets = {}

                    def loc0_of(st):
                        # first live column (within the chunk) of block st
                        j = st - (n_st - 4)
                        return 128 * j if j > 0 else 0

                    def emit_score(h, st):
                        loc0 = loc0_of(st)
                        sps = psS.tile([128, QC], F32, tag="ps")
                        nc.tensor.matmul(
                            sps[:, loc0:QC],
                            KT[:, h, st * 128:(st + 1) * 128],
                            QT[:, h, Q0 + loc0:Q0 + QC],
                            start=True, stop=True)
                        et = apool.tile([128, QC], BF16, tag="et")
                        nc.scalar.activation(et[:, loc0:QC], sps[:, loc0:QC],
                                             AF.Exp, scale=SCALE)
                        if st * 128 >= Q0:  # diagonal block: zero where s > q
                            nc.vector.tensor_mul(et[:, loc0:QC], et[:, loc0:QC],
                                                 mask[:, 0:QC - loc0])
                        ets[(h, st)] = et

                    def emit_av(h, st, yps):
                        loc0 = loc0_of(st)
                        et = ets.pop((h, st))
                        nc.tensor.matmul(
                            yps[:, loc0:QC],
                            V[:, st, h * D:(h + 1) * D],
                            et[:, loc0:QC],
                            start=(st == 0), stop=(st == n_st - 1),
                            skip_group_check=True)
                        nc.tensor.matmul(
                            dps[32 * h:32 * h + 1, loc0:QC],
                            ones[:],
                            et[:, loc0:QC],
                            start=(st == 0), stop=(st == n_st - 1),
                            skip_group_check=True)

                    for h in range(NH):
                        yps = psY.tile([128, QC], F32, tag="yps")
                        for pf in range(min(3, n_st)):
                            emit_score(h, pf)
                        for st in range(n_st):
                            if st + 3 < n_st:
                                emit_score(h, st + 3)
                            emit_av(h, st, yps)
                        if h == 0 and pending is not None:
                            # previous chunk's projection: its yTc normalize
                            # latency hides under this chunk's score matmuls
                            emit_proj(*pending)
                            pending = None
                        # park unnormalized y in SBUF so the psum bank frees
                        nc.vector.tensor_copy(yU[:, h, :], yps[:])

                    # softmax denominators: heads live at partitions 0/32/64;
                    # one reciprocal covers all three (rows between are unused
                    # garbage, reciprocal of garbage is harmless).
                    rc = accpool.tile([128, QC], BF16, tag="rc")
                    with nc.allow_low_precision(reason="bf16 softmax denom"):
                        nc.vector.reciprocal(rc[0:65, :], dps[0:65, :])
                    yTc = ypool.tile([128, NH, QC], BF16, tag="yT")  # [d, h, q]
                    for h in range(NH):
                        # partition_broadcast is only safe from base partition
                        # 0 — stage each head's row there first
                        rch = accpool.tile([1, QC], BF16, tag="rch")
                        nc.vector.tensor_copy(rch[:], rc[32 * h:32 * h + 1, :])
                        rbc = accpool.tile([128, QC], BF16, tag="rbc")
                        nc.gpsimd.partition_broadcast(rbc[:], rch[:])
                        nc.vector.tensor_mul(yTc[:, h, :], yU[:, h, :], rbc[:])

                    pending = (qc, yTc)
                emit_proj(*pending)

    nc.compile()
    return nc


def _get_nc():
    if "nc" not in _CACHE:
        _CACHE["nc"] = _build_nc()
    return _CACHE["nc"]


def _in_maps(x, cos, sin, wq, wk, wv, wproj):
    bf = ml_dtypes.bfloat16
    cosb = np.ascontiguousarray(np.asarray(cos).astype(bf))
    sinb = np.ascontiguousarray(np.asarray(sin).astype(bf))
    maskC = np.ascontiguousarray(
        (np.arange(QC)[None, :] >= np.arange(128)[:, None]).astype(bf))
    ident = np.eye(128, dtype=bf)
    maps = []
    for c in range(8):
        b = c // 2
        hs = (c % 2) * NH
        sl = slice(hs * D, (hs + NH) * D)
        maps.append({
            "xT": np.ascontiguousarray(np.asarray(x[b]).T.astype(bf)),
            "wqT": np.ascontiguousarray(np.asarray(wq)[sl].T.astype(bf)),
            "wkT": np.ascontiguousarray(np.asarray(wk)[sl].T.astype(bf)),
            "wvT": np.ascontiguousarray(np.asarray(wv)[sl].T.astype(bf)),
            "wpT": np.ascontiguousarray(np.asarray(wproj).T[sl].astype(bf)),
            "cosb": cosb,
            "sinb": sinb,
            "maskC": maskC,
            "ident": ident,
            "ones_in": np.ones((128, 1), dtype=bf),
        })
    return maps


def kernel(x, cos, sin, wq, wk, wv, wproj, _trace=False):
    nc = _get_nc()
    maps = _in_maps(x, cos, sin, wq, wk, wv, wproj)
    res = run_bass_kernel_spmd(nc, maps, core_ids=list(range(8)), trace=_trace)
    parts = [r["out"] for r in res.results]
    outv = np.stack([parts[2 * b] + parts[2 * b + 1] for b in range(B)]).astype(np.float32)
    if _trace:
        _CACHE["last_results"] = res
    return outv
